# revision 83
# baseline (speedup 1.0000x reference)
"""Trainium2 Bass kernel for nn_MemTransformerLM (hourglass Transformer-XL).

Sharding: 8 cores = 4 batch rows x 2-way tensor parallel (heads / d_inner /
vocab halves). Both cores of a pair hold the full residual stream; per-layer
partial outputs (o-proj, ff2) are summed with AllReduce-2 over core pairs.

Activations flow transposed ("T-layout": model dim on partitions, tokens on
the free axis). Weights are host-pre-transposed to contraction-major bf16.
The Transformer-XL rel_shift runs on GPSIMD local_scatter (per-partition
staircase indices; negative index = causal drop). Softmax skips the max
subtraction (scores provably small); denominators fall out of the Exp
activation's accum_out during PSUM eviction.
"""
import os
import sys
sys.path.insert(0, '/opt/trn_rl_repo')

import numpy as np
import ml_dtypes

import concourse.bass as bass
import concourse.tile as tile
from concourse import bacc, mybir
from concourse.bass_utils import run_bass_kernel_spmd

F32 = mybir.dt.float32
F32R = mybir.dt.float32r
BF16 = mybir.dt.bfloat16
I16 = mybir.dt.int16
AF = mybir.ActivationFunctionType
ALU = mybir.AluOpType

T, B, D, H, DH, DI, V, L = 1024, 4, 512, 8, 64, 2048, 256, 8
STAGES = (2, 4, 2)
S = 256
SP = 272          # padded short length (2 full tiles + 16; real rows 0..256)
KD = D // 128     # 4 d-tiles
HO = H // 2       # 4 own heads per core
QC = HO * DH      # 256 own q/k/v columns
DIO = DI // 2     # 1024 own ff-inner dims
NKI = DIO // 128  # 8 ff-inner k-tiles
NEG = -1.0e30
SCALE = 0.125

N_CORES = 8
REPS = int(os.environ.get('KERNEL_REPS', '1'))
SIM_MODE = os.environ.get('KERNEL_SIM', '0') == '1'


def ts(i, n=128):
    return slice(i * n, (i + 1) * n)


def chunk_list(NT):
    return [(c * 512, min(512, NT - c * 512)) for c in range((NT + 511) // 512)]


def _ln(nc, p, psum, xpre, NT, g, b):
    """LayerNorm over the partition (d) axis in T-layout.
    Stats via ones-matmuls; returns (XF fp32 tiles, XB bf16 tiles)."""
    chunks = chunk_list(NT)
    ones_b = _ln.ones_b
    ones_row_b = _ln.ones_row_b
    ab = p.tile([1, 2 * NT], BF16, tag="arow")
    arow = ab[:, 0:NT]
    brow = ab[:, NT:2 * NT]
    for c0 in range(0, NT, 256):
        cw = min(256, NT - c0)
        ps1 = psum.tile([1, 512], F32, tag="st1")[:, :2 * cw]
        for m in range(KD):
            # pack x (bf16) and x^2 contiguously; one matmul per m yields
            # [sums | sqsums] in a single psum bank
            xs = p.tile([128, 512], BF16, tag="xbf")[:, :2 * cw]
            nc.vector.tensor_copy(xs[:, 0:cw], xpre[m][:, c0:c0 + cw])
            nc.scalar.activation(xs[:, cw:2 * cw], xpre[m][:, c0:c0 + cw],
                                 AF.Square)
            nc.tensor.matmul(ps1, ones_b, xs, start=(m == 0), stop=(m == KD - 1))
        st = p.tile([1, 1536], F32, tag="mean")
        mean = st[:, 0:512][:, :cw]
        var = st[:, 512:1024][:, :cw]
        rv = st[:, 1024:1536][:, :cw]
        nc.vector.tensor_scalar_mul(mean, ps1[:, 0:cw], 1.0 / D)
        nc.vector.tensor_scalar_mul(var, ps1[:, cw:2 * cw], 1.0 / D)
        nc.vector.tensor_tensor(rv, mean, mean, ALU.mult)
        nc.vector.tensor_tensor(var, var, rv, ALU.subtract)
        nc.vector.tensor_scalar_add(var, var, 1.0e-5)
        nc.vector.reciprocal(rv, var)
        nc.scalar.activation(arow[:, c0:c0 + cw], rv, AF.Sqrt)
        nc.vector.tensor_tensor(brow[:, c0:c0 + cw], mean, arow[:, c0:c0 + cw], ALU.mult)
        nc.vector.tensor_scalar_mul(brow[:, c0:c0 + cw], brow[:, c0:c0 + cw], -1.0)
    XF = [p.tile([128, NT], F32, tag="XF", name=f"XF{m}") for m in range(KD)]
    XB = [p.tile([128, NT], BF16, tag="XB", name=f"XB{m}") for m in range(KD)]
    for c0, cw in chunks:
        aps = psum.tile([128, 512], F32, tag="sc")[:, :cw]
        bps = psum.tile([128, 512], F32, tag="sc")[:, :cw]
        nc.tensor.matmul(aps, ones_row_b[0:1, :], arow[:, c0:c0 + cw],
                         start=True, stop=True)
        nc.tensor.matmul(bps, ones_row_b[0:1, :], brow[:, c0:c0 + cw],
                         start=True, stop=True)
        for m in range(KD):
            t1 = p.tile([128, 512], F32, tag="lnt")[:, :cw]
            nc.vector.tensor_tensor(t1, xpre[m][:, c0:c0 + cw], aps, ALU.mult)
            nc.vector.tensor_tensor(t1, t1, bps, ALU.add)
            nc.scalar.activation(XF[m][:, c0:c0 + cw], t1, AF.Identity,
                                 bias=b[:, m:m + 1], scale=g[:, m:m + 1])
    for m in range(KD):
        nc.vector.tensor_copy(XB[m][:], XF[m][:])
    return XF, XB


def _allreduce(nc, p, dram, tin, n, NT, tag):
    """Pairwise AllReduce of a [128, n, NT] bf16 partial-sum tile, split into
    token-halves so the LN/FFN consumer of half 0 overlaps the collective for
    half 1. Returns a [128, n, NT] bf16 tile of sums."""
    for hi, (c0, cw) in enumerate(chunk_list(NT)):
        bin_ = dram.tile([128, n, cw], F32, tag=f"ari_{tag}_{hi}")
        bout = dram.tile([128, n, cw], F32, tag=f"aro_{tag}_{hi}")
        nc.sync.dma_start(bin_[:], tin[:, :, c0:c0 + cw])
        if SIM_MODE:
            # stand-in for the pair AllReduce: dram->dram copy of same bytes
            nc.sync.dma_start(bout[:], bin_[:])
        else:
            nc.gpsimd.collective_compute(
                "AllReduce", ALU.add,
                replica_groups=[[0, 1], [2, 3], [4, 5], [6, 7]],
                ins=[bin_.opt()], outs=[bout.opt()])
        # land the sums back into the partial-sum tile (dead after bin_ dma)
        nc.sync.dma_start(tin[:, :, c0:c0 + cw], bout[:])
    return tin


def ttl(NT):
    return [(i, min(128, NT - i)) for i in range(0, NT, 128)]


def _layer(nc, pools, lw, XF, XB, NT, consts):
    p, psum, dram = pools
    chunks = chunk_list(NT)
    idbf, idf, sinT, rwb, rrb = (consts[k] for k in ('idbf', 'idf', 'sinT', 'rwb', 'rrb'))

    # --- uT: own-head rk projection against the position-sinusoid table ---
    uT = []
    for m in range(2):
        u = p.tile([128, NT], BF16, tag="uT")
        for c0, cw in chunks:
            ps = psum.tile([128, 512], F32, tag="sc")[:, :cw]
            for kd in range(KD):
                nc.tensor.matmul(ps, lw['wrkT'][kd][:, ts(m)], sinT[kd][:, c0:c0 + cw],
                                 start=(kd == 0), stop=(kd == KD - 1))
            nc.scalar.copy(u[:, c0:c0 + cw], ps)
        uT.append(u)

    # --- qkv projections (q/k in T-layout; v in N-layout) ---
    qac, qbd, kb = [], [], []
    for m in range(2):
        qa = p.tile([128, NT], BF16, tag="qac")
        qb = p.tile([128, NT], BF16, tag="qbd")
        kk = p.tile([128, NT], BF16, tag="kb")
        for c0, cw in chunks:
            ps = psum.tile([128, 512], F32, tag="sc")[:, :cw]
            for kd in range(KD):
                nc.tensor.matmul(ps, lw['wqkvT'][kd][:, ts(m)], XB[kd][:, c0:c0 + cw],
                                 start=(kd == 0), stop=(kd == KD - 1))
            nc.scalar.activation(qa[:, c0:c0 + cw], ps, AF.Identity, bias=rwb[:, m:m + 1])
            nc.scalar.activation(qb[:, c0:c0 + cw], ps, AF.Identity, bias=rrb[:, m:m + 1])
            ps2 = psum.tile([128, 512], F32, tag="sc")[:, :cw]
            for kd in range(KD):
                nc.tensor.matmul(ps2, lw['wqkvT'][kd][:, ts(m + 2)], XB[kd][:, c0:c0 + cw],
                                 start=(kd == 0), stop=(kd == KD - 1))
            nc.scalar.copy(kk[:, c0:c0 + cw], ps2)
        qac.append(qa)
        qbd.append(qb)
        kb.append(kk)

    # v in N-layout with per-head 65-wide blocks: cols [65h, 65h+64) hold v,
    # col 65h+64 holds ones so the pv matmul emits softmax denominators as
    # psum row 64 for free.
    # v in N-layout, 65-wide per-head blocks: cols [65h, 65h+64) hold v and
    # col 65h+64 holds ones, so one pv matmul also emits the softmax
    # denominator as psum row 64. Plain contiguous copies/memsets only.
    vb = []
    for tt, (t0, tw) in enumerate(ttl(NT)):
        v = p.tile([128, HO * 65], BF16, tag="vb", name=f"vb{tt}")
        ps = psum.tile([128, 512], F32, tag="sc")[:tw, :QC]
        for kd in range(KD):
            nc.tensor.matmul(ps, XB[kd][:, t0:t0 + tw], lw['wqkvT'][kd][:, 512:768],
                             start=(kd == 0), stop=(kd == KD - 1))
        for h in range(HO):
            nc.vector.tensor_copy(v[:tw, h * 65:h * 65 + 64],
                                  ps[:, h * 64:h * 64 + 64])
            nc.gpsimd.memset(v[:tw, h * 65 + 64:h * 65 + 65], 1.0)
        vb.append(v)

    # --- attention: qi outer, head inner; scores kept causal.
    # Shifted-BD tiles are transposed on the PE (matmul vs identity) into the
    # same PSUM where AC^T accumulates, so prob^T comes out of one Exp pass
    # and no DMA transposes are needed. Softmax denominators ride along as a
    # ones-row matmul; the 1/den division is deferred per head to one pass.
    voT = [p.tile([128, NT], BF16, tag="voT", name=f"voT{m}") for m in range(2)]
    deninv = [p.tile([65, NT], BF16, tag="deninv", name=f"dvi{m}")
              for m in range(2)]
    ones_row_b = _ln.ones_row_b
    ones_b = _ln.ones_b
    qtiles = ttl(NT)
    for qi, (i0, qw) in enumerate(qtiles):
        W = i0 + qw
        ktiles = qtiles[:qi + 1]
        # key-tile jt occupies free block [jt*qw, jt*qw+qw) of exT (kw rows)
        idx = p.tile([128, 1024], I16, tag="idx")
        nc.gpsimd.iota(idx[:qw, :W], pattern=[[-1, W]], base=i0, channel_multiplier=1)
        wch = [(c0, min(cw, W - c0)) for c0, cw in chunks if c0 < W]
        for hh in range(HO):
            mi, po = hh // 2, (hh % 2) * 64
            pvh = psum.tile([65, 128], F32, tag="pv")[:, :qw]
            bdd = p.tile([128, 1024], BF16, tag="bdd")
            for c0, cw in wch:
                ps = psum.tile([128, 512], F32, tag="bd")[:qw, :cw]
                nc.tensor.matmul(ps, qbd[mi][po:po + 64, i0:i0 + qw],
                                 uT[mi][po:po + 64, c0:c0 + cw], start=True, stop=True)
                nc.scalar.copy(bdd[:qw, c0:c0 + cw], ps)
            bds = p.tile([128, 1024], BF16, tag="bds")
            nc.gpsimd.local_scatter(bds[:qw, :W], bdd[:qw, :W], idx[:qw, :W],
                                    channels=qw, num_elems=W, num_idxs=W)
            nc.gpsimd.affine_select(bds[:qw, i0:W], bds[:qw, i0:W],
                                    pattern=[[-1, qw]],
                                    compare_op=ALU.is_ge, fill=NEG,
                                    base=0, channel_multiplier=1)
            exT = p.tile([128, 1024], BF16, tag="ex")
            for bl0 in range(0, qi + 1, 4):
                bln = min(4, qi + 1 - bl0)
                pst = psum.tile([128, 512], F32, tag="exps")
                for jt in range(bl0, bl0 + bln):
                    j0, kw = ktiles[jt]
                    b0 = (jt - bl0) * qw
                    nc.tensor.matmul(pst[:kw, b0:b0 + qw], bds[:qw, j0:j0 + kw],
                                     idbf[:qw, :qw], start=True, stop=False)
                    nc.tensor.matmul(pst[:kw, b0:b0 + qw],
                                     kb[mi][po:po + 64, j0:j0 + kw],
                                     qac[mi][po:po + 64, i0:i0 + qw],
                                     start=False, stop=True)
                nfull = sum(1 for jt in range(bl0, bl0 + bln)
                            if ktiles[jt][1] == 128)
                if nfull:
                    nc.scalar.activation(
                        exT[:, bl0 * qw:(bl0 + nfull) * qw],
                        pst[:, 0:nfull * qw], AF.Exp, scale=SCALE)
                for jt in range(bl0 + nfull, bl0 + bln):
                    j0, kw = ktiles[jt]
                    b0 = (jt - bl0) * qw
                    nc.scalar.activation(exT[:kw, jt * qw:jt * qw + qw],
                                         pst[:kw, b0:b0 + qw], AF.Exp, scale=SCALE)
            for jt in range(qi + 1):
                j0, kw = ktiles[jt]
                nc.tensor.matmul(pvh, vb[jt][:kw, hh * 65:hh * 65 + 65],
                                 exT[:kw, jt * qw:jt * qw + qw],
                                 start=(jt == 0), stop=(jt == qi))
            with nc.allow_low_precision(reason="softmax 1/den rounds to bf16"):
                nc.vector.reciprocal(deninv[mi][po:po + 1, i0:i0 + qw],
                                     pvh[64:65, :])
            vtmp = p.tile([64, 128], BF16, tag="dbb")[:, :qw]
            nc.vector.tensor_copy(vtmp, pvh[0:64, :])
            bps = psum.tile([128, 512], F32, tag="bd")[:64, :qw]
            nc.tensor.matmul(bps, ones_row_b[po:po + 1, :64],
                             deninv[mi][po:po + 1, i0:i0 + qw],
                             start=True, stop=True)
            nc.vector.tensor_tensor(voT[mi][po:po + 64, i0:i0 + qw],
                                    vtmp, bps, ALU.mult)

    # --- o-proj partial + AllReduce + residual + LN1 ---
    ob = p.tile([128, KD, NT], F32, tag="arin", name="ob")
    for m in range(KD):
        for c0, cw in chunks:
            ps = psum.tile([128, 512], F32, tag="sc")[:, :cw]
            for kt in range(2):
                nc.tensor.matmul(ps, lw['woT'][kt][:, ts(m)], voT[kt][:, c0:c0 + cw],
                                 start=(kt == 0), stop=(kt == 1))
            nc.vector.tensor_copy(ob[:, m, c0:c0 + cw], ps)
    oar = _allreduce(nc, p, dram, ob, KD, NT, tag=f"o{lw['li']}_{lw['rep']}")
    xpre = []
    for m in range(KD):
        xp = p.tile([128, NT], F32, tag="big", name=f"xp{m}")
        nc.vector.tensor_tensor(xp[:], oar[:, m], XF[m][:], ALU.add)
        xpre.append(xp)
    XF, XB = _ln(nc, p, psum, xpre, NT, lw['g1'], lw['bb1'])

    # --- FFN ---
    hb = []
    for m in range(NKI):
        hbt = p.tile([128, NT], BF16, tag="hb", name=f"hb{m}")
        for c0, cw in chunks:
            ps = psum.tile([128, 512], F32, tag="sc")[:, :cw]
            for kd in range(KD):
                nc.tensor.matmul(ps, lw['w1T'][kd][:, ts(m)], XB[kd][:, c0:c0 + cw],
                                 start=(kd == 0), stop=(kd == KD - 1))
            nc.scalar.activation(hbt[:, c0:c0 + cw], ps, AF.Relu,
                                 bias=lw['fb1'][:, m:m + 1])
        hb.append(hbt)
    fb = p.tile([128, KD, NT], F32, tag="arin", name="fb")
    for m in range(KD):
        for c0, cw in chunks:
            ps = psum.tile([128, 512], F32, tag="sc")[:, :cw]
            for kt in range(NKI):
                nc.tensor.matmul(ps, lw['w2T'][kt][:, ts(m)], hb[kt][:, c0:c0 + cw],
                                 start=(kt == 0), stop=(kt == NKI - 1))
            nc.vector.tensor_copy(fb[:, m, c0:c0 + cw], ps)
    far = _allreduce(nc, p, dram, fb, KD, NT, tag=f"f{lw['li']}_{lw['rep']}")
    xpre2 = []
    for m in range(KD):
        xp = p.tile([128, NT], F32, tag="big", name=f"xq{m}")
        for c0, cw in chunks:
            t1 = p.tile([128, 512], F32, tag="lnt")[:, :cw]
            nc.scalar.activation(t1, far[:, m, c0:c0 + cw], AF.Identity,
                                 bias=lw['fb2'][:, m:m + 1])
            nc.vector.tensor_tensor(xp[:, c0:c0 + cw], t1,
                                    XF[m][:, c0:c0 + cw], ALU.add)
        xpre2.append(xp)
    return _ln(nc, p, psum, xpre2, NT, lw['g2'], lw['bb2'])


def build_program():
    nc = bacc.Bacc("TRN2", target_bir_lowering=False, debug=False, num_devices=N_CORES)
    d = {}

    def di(name, shape, dt):
        d[name] = nc.dram_tensor(name, shape, dt, kind="ExternalInput")

    di("wqkvT", [L, D, 3 * QC], BF16)
    di("wrkT", [L, D, QC], BF16)
    di("woT", [L, QC, D], BF16)
    di("w1T", [L, D, DIO], BF16)
    di("w2T", [L, DIO, D], BF16)
    di("biases", [L, 128, 28], F32)
    di("rwb", [QC], F32)
    di("rrb", [QC], F32)
    di("wemb", [V, D], BF16)
    di("onehotT", [V, T], BF16)
    di("sinTd", [D, T], BF16)
    di("idbf", [128, 128], BF16)
    di("idf", [128, 128], F32)
    di("wpool", [T, SP], BF16)
    di("nullv", [D], F32)
    di("gd", [D], F32)
    di("bdn", [D], F32)
    di("uup", [SP, T], BF16)
    di("finT", [D, V // 2], BF16)
    di("fbn", [V // 2], F32)
    logits = nc.dram_tensor("logits", [T, V // 2], F32, kind="ExternalOutput")

    with tile.TileContext(nc) as tc:
        import itertools
        _ctr = itertools.count()

        class NP:
            def __init__(self, pool):
                self.pool = pool

            def tile(self, shape, dt, tag=None, name=None):
                if name is None:
                    name = f"{tag}_{next(_ctr)}"
                return self.pool.tile(shape, dt, tag=tag, name=name)

        with tc.tile_pool(name="p", bufs=2) as p_r, \
             tc.tile_pool(name="pbig", bufs=4) as pbig_r, \
             tc.tile_pool(name="px", bufs=4) as px_r, \
             tc.tile_pool(name="pr", bufs=4) as pr_r, \
             tc.tile_pool(name="ph", bufs=8) as ph_r, \
             tc.tile_pool(name="pw", bufs=2) as pw_r, \
             tc.tile_pool(name="pw1", bufs=2) as pw1_r, \
             tc.tile_pool(name="pw2", bufs=1) as pw2_r, \
             tc.tile_pool(name="pc", bufs=1) as pc_r, \
             tc.tile_pool(name="psum", bufs=2, space="PSUM") as psum_r, \
             tc.tile_pool(name="psum1", bufs=1, space="PSUM") as psum1_r, \
             tc.tile_pool(name="dram", bufs=1, space="DRAM") as dram_r:
            p, pbig, px, pr, ph, pw, pw1, pw2, pc = (NP(x) for x in
                                            (p_r, pbig_r, px_r, pr_r, ph_r,
                                             pw_r, pw1_r, pw2_r, pc_r))
            psum_, psum1_, dram = NP(psum_r), NP(psum1_r), NP(dram_r)

            # pool router: route tags to pools with the right bufs counts
            class P:
                def tile(self, shape, dt, tag=None, name=None):
                    if tag in ("XF", "XB"):
                        return px.tile(shape, dt, tag=tag, name=name)
                    if tag in ("big", "residF"):
                        return pbig.tile(shape, dt, tag=tag, name=name)
                    if tag in ("hb", "vb"):
                        return ph.tile(shape, dt, tag=tag, name=name)
                    if tag == "arin":
                        return pw2.tile(shape, dt, tag=tag, name=name)
                    if tag in ("arow", "brow", "mean", "var", "rv", "msq", "dro"):
                        return pc.tile(shape, dt, tag=tag, name=name)
                    if tag is not None and tag.startswith("c_"):
                        return pc.tile(shape, dt, tag=tag, name=name)
                    return p.tile(shape, dt, tag=tag, name=name)
            pp = P()

            class PS:
                def tile(self, shape, dt, tag=None, name=None):
                    if tag in ("st1", "exps"):
                        return psum1_.tile(shape, dt, tag=tag, name=name)
                    return psum_.tile(shape, dt, tag=tag, name=name)
            pps = PS()
            pools = (pp, pps, dram)

            consts = {}
            idbf = pc.tile([128, 128], BF16, tag="c_idbf")
            nc.sync.dma_start(idbf[:], d["idbf"].ap())
            idf = None  # unused
            ones_b = pc.tile([128, 1], BF16, tag="c_ones")
            nc.gpsimd.memset(ones_b[:], 1.0)
            ones_row = pc.tile([1, 128], F32, tag="c_onesr")
            nc.gpsimd.memset(ones_row[:], 1.0)
            ones_row_b = pc.tile([65, 128], BF16, tag="c_onesrb")
            nc.gpsimd.memset(ones_row_b[:], 1.0)
            _ln.ones_b = ones_b
            _ln.ones_row = ones_row
            _ln.ones_row_b = ones_row_b
            sinT_t = pc.tile([128, KD, T], BF16, tag="c_sin")
            nc.sync.dma_start(sinT_t[:],
                              d["sinTd"].ap().rearrange("(a q) t -> q a t", q=128))
            sinT = [sinT_t[:, k] for k in range(KD)]
            rwb = pc.tile([128, 2], F32, tag="c_rwb")
            nc.sync.dma_start(rwb[:], d["rwb"].ap().rearrange("(a q) -> q a", q=128))
            rrb = pc.tile([128, 2], F32, tag="c_rrb")
            nc.sync.dma_start(rrb[:], d["rrb"].ap().rearrange("(a q) -> q a", q=128))
            consts.update(idbf=idbf, idf=idf, sinT=sinT, rwb=rwb, rrb=rrb)

            def load_layer(li, rep):
                lw = {'li': li, 'rep': rep}
                wqkv = pw.tile([128, KD, 3 * QC], BF16, tag="w_qkv")
                nc.sync.dma_start(
                    wqkv[:], d["wqkvT"].ap()[li].rearrange("(a q) o -> q a o", q=128))
                lw['wqkvT'] = [wqkv[:, k] for k in range(KD)]
                wrk = pw.tile([128, KD, QC], BF16, tag="w_rk")
                nc.sync.dma_start(
                    wrk[:], d["wrkT"].ap()[li].rearrange("(a q) o -> q a o", q=128))
                lw['wrkT'] = [wrk[:, k] for k in range(KD)]
                wo = pw.tile([128, 2, D], BF16, tag="w_o")
                nc.sync.dma_start(
                    wo[:], d["woT"].ap()[li].rearrange("(a q) o -> q a o", q=128))
                lw['woT'] = [wo[:, k] for k in range(2)]
                w1 = pw1.tile([128, KD, DIO], BF16, tag="w_1")
                nc.sync.dma_start(
                    w1[:], d["w1T"].ap()[li].rearrange("(a q) o -> q a o", q=128))
                lw['w1T'] = [w1[:, k] for k in range(KD)]
                w2 = pw2.tile([128, NKI, D], BF16, tag="w_2")
                nc.sync.dma_start(
                    w2[:], d["w2T"].ap()[li].rearrange("(a q) o -> q a o", q=128))
                lw['w2T'] = [w2[:, k] for k in range(NKI)]
                bt = pw.tile([128, 28], F32, tag="w_b")
                nc.sync.dma_start(bt[:], d["biases"].ap()[li])
                lw['fb1'] = bt[:, 0:8]
                lw['fb2'] = bt[:, 8:12]
                lw['g1'] = bt[:, 12:16]
                lw['bb1'] = bt[:, 16:20]
                lw['g2'] = bt[:, 20:24]
                lw['bb2'] = bt[:, 24:28]
                return lw

            for rep in range(REPS):
                # --- embedding (one-hot matmul) ---
                wembt = pp.tile([128, 2, D], BF16, tag="c_wembt", name="wembt")
                nc.sync.dma_start(
                    wembt[:], d["wemb"].ap().rearrange("(a q) e -> q a e", q=128))
                ohs = []
                for c in range(2):
                    oh = pp.tile([128, 2, 512], BF16, tag=f"c_oh{c}", name=f"oh{c}")
                    nc.sync.dma_start(
                        oh[:],
                        d["onehotT"].ap().rearrange("(a q) t -> q a t", q=128)
                        [:, :, c * 512:(c + 1) * 512])
                    ohs.append(oh)
                XF, XB = [], []
                for m in range(KD):
                    xf = pp.tile([128, T], F32, tag="XF", name=f"XF{m}")
                    xb = pp.tile([128, T], BF16, tag="XB", name=f"XB{m}")
                    for c in range(2):
                        ps = pps.tile([128, 512], F32, tag="sc")
                        for vk in range(2):
                            nc.tensor.matmul(ps, wembt[:, vk, ts(m)], ohs[c][:, vk],
                                             start=(vk == 0), stop=(vk == 1))
                        nc.vector.tensor_copy(xf[:, c * 512:(c + 1) * 512], ps)
                        nc.scalar.copy(xb[:, c * 512:(c + 1) * 512], ps)
                    XF.append(xf)
                    XB.append(xb)

                for li in range(STAGES[0]):
                    XF, XB = _layer(nc, pools, load_layer(li, rep), XF, XB, T, consts)

                residF = []
                for m in range(KD):
                    r = pbig.tile([128, T], BF16, tag="residF", name=f"res{m}")
                    nc.vector.tensor_copy(r[:], XF[m][:])
                    residF.append(r)

                # --- downsample ---
                XN = []
                for tt in range(T // 128):
                    xn = pp.tile([128, D], BF16, tag="hb", name=f"xn{tt}")
                    for m in range(KD):
                        pt = pps.tile([128, 128], BF16, tag="sc")
                        nc.tensor.transpose(pt[:], XB[m][:, ts(tt)], idbf)
                        nc.vector.tensor_copy(xn[:, ts(m)], pt[:])
                    XN.append(xn)
                wpool = [pp.tile([128, SP], BF16, tag="vb", name=f"pl{tt}")
                         for tt in range(T // 128)]
                for tt in range(T // 128):
                    nc.sync.dma_start(wpool[tt][:], d["wpool"].ap()[ts(tt), :])
                nullv = pc.tile([128, KD], F32, tag="c_null")
                nc.sync.dma_start(nullv[:], d["nullv"].ap().rearrange("(a q) -> q a", q=128))
                spre = []
                for m in range(KD):
                    sp_ = pbig.tile([128, SP], F32, tag="big", name=f"sp{m}")
                    ps = pps.tile([128, 512], F32, tag="sc")[:, :SP]
                    for tt in range(T // 128):
                        nc.tensor.matmul(ps, XN[tt][:, ts(m)], wpool[tt][:],
                                         start=(tt == 0), stop=(tt == T // 128 - 1))
                    nc.vector.tensor_copy(sp_[:], ps)
                    nc.vector.tensor_copy(sp_[:, 0:1], nullv[:, m:m + 1])
                    spre.append(sp_)
                gdt = pc.tile([128, KD], F32, tag="c_gd")
                nc.sync.dma_start(gdt[:], d["gd"].ap().rearrange("(a q) -> q a", q=128))
                bdt = pc.tile([128, KD], F32, tag="c_bd")
                nc.sync.dma_start(bdt[:], d["bdn"].ap().rearrange("(a q) -> q a", q=128))
                SXF, SXB = _ln(nc, pp, pps, spre, SP, gdt, bdt)

                for li in range(STAGES[0], STAGES[0] + STAGES[1]):
                    SXF, SXB = _layer(nc, pools, load_layer(li, rep), SXF, SXB, SP, consts)

                # --- upsample + residual ---
                stiles = ttl(SP)
                SN = []
                for st, (s0, sw) in enumerate(stiles):
                    sn = pp.tile([128, D], BF16, tag="hb", name=f"sn{st}")
                    for m in range(KD):
                        pt = pps.tile([128, 128], BF16, tag="sc")[:sw, :]
                        nc.tensor.transpose(pt, SXB[m][:, s0:s0 + sw], idbf)
                        nc.vector.tensor_copy(sn[:sw, ts(m)], pt)
                    SN.append(sn)
                uup = [pp.tile([128, T], BF16, tag="hb", name=f"uu{st}")
                       for st in range(len(stiles))]
                for st, (s0, sw) in enumerate(stiles):
                    nc.sync.dma_start(uup[st][:sw, :], d["uup"].ap()[s0:s0 + sw, :])
                XF2, XB2 = [], []
                for m in range(KD):
                    xf = pp.tile([128, T], F32, tag="XF", name=f"XF{m}")
                    xb = pp.tile([128, T], BF16, tag="XB", name=f"XB{m}")
                    for c in range(2):
                        ps = pps.tile([128, 512], F32, tag="sc")
                        for st, (s0, sw) in enumerate(stiles):
                            nc.tensor.matmul(ps, SN[st][:sw, ts(m)],
                                             uup[st][:sw, c * 512:(c + 1) * 512],
                                             start=(st == 0),
                                             stop=(st == len(stiles) - 1))
                        nc.vector.tensor_tensor(xf[:, c * 512:(c + 1) * 512], ps,
                                                residF[m][:, c * 512:(c + 1) * 512], ALU.add)
                        nc.scalar.copy(xb[:, c * 512:(c + 1) * 512],
                                       xf[:, c * 512:(c + 1) * 512])
                    XF2.append(xf)
                    XB2.append(xb)
                XF, XB = XF2, XB2

                for li in range(STAGES[0] + STAGES[1], L):
                    XF, XB = _layer(nc, pools, load_layer(li, rep), XF, XB, T, consts)

                # --- final vocab projection (own half) ---
                finTt = pp.tile([128, KD, V // 2], BF16, tag="c_finT", name="finTt")
                nc.sync.dma_start(
                    finTt[:], d["finT"].ap().rearrange("(a q) v -> q a v", q=128))
                finT = [finTt[:, k] for k in range(KD)]
                fbn = pc.tile([1, V // 2], F32, tag="c_fbn")
                nc.sync.dma_start(fbn[:], d["fbn"].ap()[None, :])
                for tt in range(T // 128):
                    ps = pps.tile([128, 512], F32, tag="sc")[:, :V // 2]
                    for kd in range(KD):
                        nc.tensor.matmul(ps, XB[kd][:, ts(tt)], finT[kd][:],
                                         start=(kd == 0), stop=(kd == KD - 1))
                    nc.tensor.matmul(ps, ones_row, fbn[:], start=False, stop=True)
                    lo = p.tile([128, V // 2], F32, tag="louts")
                    nc.vector.tensor_copy(lo[:], ps)
                    nc.sync.dma_start(logits.ap()[ts(tt), :], lo[:])

    nc.compile()
    return nc


def host_inputs(inputs):
    bf = lambda x: np.ascontiguousarray(np.asarray(x, dtype=np.float32)).astype(ml_dtypes.bfloat16)
    f32 = lambda x: np.ascontiguousarray(np.asarray(x), dtype=np.float32)
    qkv_w = f32(inputs['qkv_w'])
    rk_w = f32(inputs['rk_w'])
    o_w = f32(inputs['o_w'])
    ff_w1 = f32(inputs['ff_w1'])
    ff_w2 = f32(inputs['ff_w2'])
    data = np.asarray(inputs['data'])
    bnd = np.asarray(inputs['boundaries_gt'])

    inv = 1.0 / (10000.0 ** (np.arange(0, D, 2, dtype=np.float32) / D))
    ang = np.arange(T, dtype=np.float32)[:, None] * inv[None, :]
    sin_tab = np.concatenate([np.sin(ang), np.cos(ang)], -1).astype(np.float32)
    eye = np.eye(128, dtype=np.float32)

    in_maps = []
    for c in range(N_CORES):
        b, h = c // 2, c % 2
        heads = list(range(h * HO, h * HO + HO))
        qr = np.concatenate([np.arange(g * DH, (g + 1) * DH) for g in heads])
        di_own = np.arange(h * DIO, (h + 1) * DIO)
        v_own = np.arange(h * (V // 2), (h + 1) * (V // 2))

        im = {}
        im['wqkvT'] = bf(np.stack([qkv_w[l][np.concatenate([qr, 512 + qr, 1024 + qr])].T
                                   for l in range(L)]))
        im['wrkT'] = bf(np.stack([rk_w[l][qr].T for l in range(L)]))
        im['woT'] = bf(np.stack([o_w[l][:, qr].T for l in range(L)]))
        im['w1T'] = bf(np.stack([ff_w1[l][di_own].T for l in range(L)]))
        im['w2T'] = bf(np.stack([ff_w2[l][:, di_own].T for l in range(L)]))
        fb1 = f32(np.asarray(inputs['ff_b1'])[:, di_own])        # [L, 1024]
        vecs = [f32(inputs['ff_b2']), f32(inputs['ln1_g']), f32(inputs['ln1_b']),
                f32(inputs['ln2_g']), f32(inputs['ln2_b'])]      # [L, 512] each
        # column q, slot a holds v[a*128+q] (matches "(a q) -> q a")
        bias_pack = np.zeros((L, 128, 28), np.float32)
        bias_pack[:, :, 0:8] = fb1.reshape(L, 8, 128).transpose(0, 2, 1)
        for i, v in enumerate(vecs):
            bias_pack[:, :, 8 + 4 * i:12 + 4 * i] = \
                v.reshape(L, 4, 128).transpose(0, 2, 1)
        im['biases'] = bias_pack
        im['rwb'] = f32(np.asarray(inputs['r_w_bias'])[heads].reshape(-1))
        im['rrb'] = f32(np.asarray(inputs['r_r_bias'])[heads].reshape(-1))
        im['wemb'] = bf(inputs['word_emb'])
        oh = np.zeros((V, T), np.float32)
        oh[data[:, b], np.arange(T)] = 1.0
        im['onehotT'] = bf(oh)
        im['sinTd'] = bf(sin_tab.T)
        im['idbf'] = bf(eye)
        im['idf'] = f32(eye)
        hard = bnd[:, b].astype(np.float32)
        seg = np.cumsum(hard) - hard
        ind = (seg[:, None] == np.arange(S)).astype(np.float32)
        wmat = ind / (ind.sum(0, keepdims=True) + 1e-9)
        wp_ = np.zeros((T, SP), np.float32)     # cols S+1..SP stay zero pad
        wp_[:, 1:1 + S] = wmat
        im['wpool'] = bf(wp_)
        im['nullv'] = f32(np.asarray(inputs['null_group']).reshape(-1))
        im['gd'] = f32(inputs['down_ln_g'])
        im['bdn'] = f32(inputs['down_ln_b'])
        segU = np.clip(np.cumsum(hard).astype(np.int64), 0, S)
        uu = np.zeros((SP, T), np.float32)
        uu[segU, np.arange(T)] = 1.0
        im['uup'] = bf(uu)
        im['finT'] = bf(f32(inputs['final_w'])[v_own].T)
        im['fbn'] = f32(np.asarray(inputs['final_b'])[v_own])
        in_maps.append(im)
    return in_maps


_NC_CACHE = {}


def get_program():
    if 'nc' not in _NC_CACHE:
        _NC_CACHE['nc'] = build_program()
    return _NC_CACHE['nc']


def assemble_output(results) -> np.ndarray:
    out = np.zeros((T, B, V), np.float32)
    for c in range(N_CORES):
        b, h = c // 2, c % 2
        out[:, b, h * (V // 2):(h + 1) * (V // 2)] = results[c]['logits']
    return out


def kernel(**inputs) -> np.ndarray:
    nc = get_program()
    in_maps = host_inputs(inputs)
    res = run_bass_kernel_spmd(nc, in_maps, core_ids=list(range(N_CORES)), trace=False)
    return assemble_output(res.results)



# revision 86
# speedup vs baseline: 1.0356x; 1.0356x over previous
"""Trainium2 Bass kernel for nn_MemTransformerLM (hourglass Transformer-XL).

Sharding: 8 cores = 4 batch rows x 2-way tensor parallel (heads / d_inner /
vocab halves). Both cores of a pair hold the full residual stream; per-layer
partial outputs (o-proj, ff2) are summed with AllReduce-2 over core pairs.

Activations flow transposed ("T-layout": model dim on partitions, tokens on
the free axis). Weights are host-pre-transposed to contraction-major bf16.
The Transformer-XL rel_shift runs on GPSIMD local_scatter (per-partition
staircase indices; negative index = causal drop). Softmax skips the max
subtraction (scores provably small); denominators fall out of the Exp
activation's accum_out during PSUM eviction.
"""
import os
import sys
sys.path.insert(0, '/opt/trn_rl_repo')

import numpy as np
import ml_dtypes

import concourse.bass as bass
import concourse.tile as tile
from concourse import bacc, mybir
from concourse.bass_utils import run_bass_kernel_spmd

F32 = mybir.dt.float32
F32R = mybir.dt.float32r
BF16 = mybir.dt.bfloat16
I16 = mybir.dt.int16
AF = mybir.ActivationFunctionType
ALU = mybir.AluOpType

T, B, D, H, DH, DI, V, L = 1024, 4, 512, 8, 64, 2048, 256, 8
STAGES = (2, 4, 2)
S = 256
SP = 272          # padded short length (2 full tiles + 16; real rows 0..256)
KD = D // 128     # 4 d-tiles
HO = H // 2       # 4 own heads per core
QC = HO * DH      # 256 own q/k/v columns
DIO = DI // 2     # 1024 own ff-inner dims
NKI = DIO // 128  # 8 ff-inner k-tiles
NEG = -1.0e30
SCALE = 0.125

N_CORES = 8
REPS = int(os.environ.get('KERNEL_REPS', '1'))
SIM_MODE = os.environ.get('KERNEL_SIM', '0') == '1'


def ts(i, n=128):
    return slice(i * n, (i + 1) * n)


def chunk_list(NT):
    return [(c * 512, min(512, NT - c * 512)) for c in range((NT + 511) // 512)]


def _ln(nc, p, psum, xpre, NT, g, b):
    """LayerNorm over the partition (d) axis in T-layout.
    Stats via ones-matmuls; returns (XF fp32 tiles, XB bf16 tiles)."""
    chunks = chunk_list(NT)
    ones_b = _ln.ones_b
    ones_row_b = _ln.ones_row_b
    ab = p.tile([1, 2 * NT], BF16, tag="arow")
    arow = ab[:, 0:NT]
    brow = ab[:, NT:2 * NT]
    for c0 in range(0, NT, 256):
        cw = min(256, NT - c0)
        ps1 = psum.tile([1, 512], F32, tag="st1")[:, :2 * cw]
        for m in range(KD):
            # pack x (bf16) and x^2 contiguously; one matmul per m yields
            # [sums | sqsums] in a single psum bank
            xs = p.tile([128, 512], BF16, tag="xbf")[:, :2 * cw]
            nc.vector.tensor_copy(xs[:, 0:cw], xpre[m][:, c0:c0 + cw])
            nc.scalar.activation(xs[:, cw:2 * cw], xpre[m][:, c0:c0 + cw],
                                 AF.Square)
            nc.tensor.matmul(ps1, ones_b, xs, start=(m == 0), stop=(m == KD - 1))
        st = p.tile([1, 1536], F32, tag="mean")
        mean = st[:, 0:512][:, :cw]
        var = st[:, 512:1024][:, :cw]
        rv = st[:, 1024:1536][:, :cw]
        nc.vector.tensor_scalar_mul(mean, ps1[:, 0:cw], 1.0 / D)
        nc.vector.tensor_scalar_mul(var, ps1[:, cw:2 * cw], 1.0 / D)
        nc.vector.tensor_tensor(rv, mean, mean, ALU.mult)
        nc.vector.tensor_tensor(var, var, rv, ALU.subtract)
        nc.vector.tensor_scalar_add(var, var, 1.0e-5)
        nc.vector.reciprocal(rv, var)
        nc.scalar.activation(arow[:, c0:c0 + cw], rv, AF.Sqrt)
        nc.vector.tensor_tensor(brow[:, c0:c0 + cw], mean, arow[:, c0:c0 + cw], ALU.mult)
        nc.vector.tensor_scalar_mul(brow[:, c0:c0 + cw], brow[:, c0:c0 + cw], -1.0)
    XF = [p.tile([128, NT], F32, tag="XF", name=f"XF{m}") for m in range(KD)]
    XB = [p.tile([128, NT], BF16, tag="XB", name=f"XB{m}") for m in range(KD)]
    for c0, cw in chunks:
        aps = psum.tile([128, 512], F32, tag="sc")[:, :cw]
        bps = psum.tile([128, 512], F32, tag="sc")[:, :cw]
        nc.tensor.matmul(aps, ones_row_b[0:1, :], arow[:, c0:c0 + cw],
                         start=True, stop=True)
        nc.tensor.matmul(bps, ones_row_b[0:1, :], brow[:, c0:c0 + cw],
                         start=True, stop=True)
        for m in range(KD):
            t1 = p.tile([128, 512], F32, tag="lnt")[:, :cw]
            nc.vector.tensor_tensor(t1, xpre[m][:, c0:c0 + cw], aps, ALU.mult)
            nc.vector.tensor_tensor(t1, t1, bps, ALU.add)
            nc.scalar.activation(XF[m][:, c0:c0 + cw], t1, AF.Identity,
                                 bias=b[:, m:m + 1], scale=g[:, m:m + 1])
    for m in range(KD):
        nc.vector.tensor_copy(XB[m][:], XF[m][:])
    return XF, XB


def _allreduce(nc, p, dram, tin, n, NT, tag):
    """Pairwise AllReduce of a [128, n, NT] bf16 partial-sum tile, split into
    token-halves so the LN/FFN consumer of half 0 overlaps the collective for
    half 1. Returns a [128, n, NT] bf16 tile of sums."""
    for hi, (c0, cw) in enumerate(chunk_list(NT)):
        bin_ = dram.tile([128, n, cw], BF16, tag=f"ari_{tag}_{hi}")
        bout = dram.tile([128, n, cw], BF16, tag=f"aro_{tag}_{hi}")
        nc.sync.dma_start(bin_[:], tin[:, :, c0:c0 + cw])
        if SIM_MODE:
            # stand-in for the pair AllReduce: dram->dram copy of same bytes
            nc.sync.dma_start(bout[:], bin_[:])
        else:
            nc.gpsimd.collective_compute(
                "AllReduce", ALU.add,
                replica_groups=[[0, 1], [2, 3], [4, 5], [6, 7]],
                ins=[bin_.opt()], outs=[bout.opt()])
        # land the sums back into the partial-sum tile (dead after bin_ dma)
        nc.sync.dma_start(tin[:, :, c0:c0 + cw], bout[:])
    return tin


def ttl(NT):
    return [(i, min(128, NT - i)) for i in range(0, NT, 128)]


def _layer(nc, pools, lw, XF, XB, NT, consts):
    p, psum, dram = pools
    chunks = chunk_list(NT)
    idbf, idf, sinT, rwb, rrb = (consts[k] for k in ('idbf', 'idf', 'sinT', 'rwb', 'rrb'))

    # --- uT: own-head rk projection against the position-sinusoid table ---
    uT = []
    for m in range(2):
        u = p.tile([128, NT], BF16, tag="uT")
        for c0, cw in chunks:
            ps = psum.tile([128, 512], F32, tag="sc")[:, :cw]
            for kd in range(KD):
                nc.tensor.matmul(ps, lw['wrkT'][kd][:, ts(m)], sinT[kd][:, c0:c0 + cw],
                                 start=(kd == 0), stop=(kd == KD - 1))
            nc.scalar.copy(u[:, c0:c0 + cw], ps)
        uT.append(u)

    # --- qkv projections (q/k in T-layout; v in N-layout) ---
    qac, qbd, kb = [], [], []
    for m in range(2):
        qa = p.tile([128, NT], BF16, tag="qac")
        qb = p.tile([128, NT], BF16, tag="qbd")
        kk = p.tile([128, NT], BF16, tag="kb")
        for c0, cw in chunks:
            ps = psum.tile([128, 512], F32, tag="sc")[:, :cw]
            for kd in range(KD):
                nc.tensor.matmul(ps, lw['wqkvT'][kd][:, ts(m)], XB[kd][:, c0:c0 + cw],
                                 start=(kd == 0), stop=(kd == KD - 1))
            nc.scalar.activation(qa[:, c0:c0 + cw], ps, AF.Identity, bias=rwb[:, m:m + 1])
            nc.scalar.activation(qb[:, c0:c0 + cw], ps, AF.Identity, bias=rrb[:, m:m + 1])
            ps2 = psum.tile([128, 512], F32, tag="sc")[:, :cw]
            for kd in range(KD):
                nc.tensor.matmul(ps2, lw['wqkvT'][kd][:, ts(m + 2)], XB[kd][:, c0:c0 + cw],
                                 start=(kd == 0), stop=(kd == KD - 1))
            nc.scalar.copy(kk[:, c0:c0 + cw], ps2)
        qac.append(qa)
        qbd.append(qb)
        kb.append(kk)

    # v in N-layout with per-head 65-wide blocks: cols [65h, 65h+64) hold v,
    # col 65h+64 holds ones so the pv matmul emits softmax denominators as
    # psum row 64 for free.
    # v in N-layout, 65-wide per-head blocks: cols [65h, 65h+64) hold v and
    # col 65h+64 holds ones, so one pv matmul also emits the softmax
    # denominator as psum row 64. Plain contiguous copies/memsets only.
    vb = []
    for tt, (t0, tw) in enumerate(ttl(NT)):
        v = p.tile([128, HO * 65], BF16, tag="vb", name=f"vb{tt}")
        ps = psum.tile([128, 512], F32, tag="sc")[:tw, :QC]
        for kd in range(KD):
            nc.tensor.matmul(ps, XB[kd][:, t0:t0 + tw], lw['wqkvT'][kd][:, 512:768],
                             start=(kd == 0), stop=(kd == KD - 1))
        for h in range(HO):
            nc.vector.tensor_copy(v[:tw, h * 65:h * 65 + 64],
                                  ps[:, h * 64:h * 64 + 64])
            nc.gpsimd.memset(v[:tw, h * 65 + 64:h * 65 + 65], 1.0)
        vb.append(v)

    # --- attention: qi outer, head inner; scores kept causal.
    # Shifted-BD tiles are transposed on the PE (matmul vs identity) into the
    # same PSUM where AC^T accumulates, so prob^T comes out of one Exp pass
    # and no DMA transposes are needed. Softmax denominators ride along as a
    # ones-row matmul; the 1/den division is deferred per head to one pass.
    voT = [p.tile([128, NT], BF16, tag="voT", name=f"voT{m}") for m in range(2)]
    deninv = [p.tile([65, NT], BF16, tag="deninv", name=f"dvi{m}")
              for m in range(2)]
    ones_row_b = _ln.ones_row_b
    ones_b = _ln.ones_b
    qtiles = ttl(NT)
    for qi, (i0, qw) in enumerate(qtiles):
        W = i0 + qw
        ktiles = qtiles[:qi + 1]
        # key-tile jt occupies free block [jt*qw, jt*qw+qw) of exT (kw rows)
        idx = p.tile([128, 1024], I16, tag="idx")
        nc.gpsimd.iota(idx[:qw, :W], pattern=[[-1, W]], base=i0, channel_multiplier=1)
        wch = [(c0, min(cw, W - c0)) for c0, cw in chunks if c0 < W]
        for hh in range(HO):
            mi, po = hh // 2, (hh % 2) * 64
            pvh = psum.tile([65, 128], F32, tag="pv")[:, :qw]
            bdd = p.tile([128, 1024], BF16, tag="bdd")
            for c0, cw in wch:
                ps = psum.tile([128, 512], F32, tag="bd")[:qw, :cw]
                nc.tensor.matmul(ps, qbd[mi][po:po + 64, i0:i0 + qw],
                                 uT[mi][po:po + 64, c0:c0 + cw], start=True, stop=True)
                nc.scalar.copy(bdd[:qw, c0:c0 + cw], ps)
            bds = p.tile([128, 1024], BF16, tag="bds")
            nc.gpsimd.local_scatter(bds[:qw, :W], bdd[:qw, :W], idx[:qw, :W],
                                    channels=qw, num_elems=W, num_idxs=W)
            nc.gpsimd.affine_select(bds[:qw, i0:W], bds[:qw, i0:W],
                                    pattern=[[-1, qw]],
                                    compare_op=ALU.is_ge, fill=NEG,
                                    base=0, channel_multiplier=1)
            exT = p.tile([128, 1024], BF16, tag="ex")
            for bl0 in range(0, qi + 1, 4):
                bln = min(4, qi + 1 - bl0)
                pst = psum.tile([128, 512], F32, tag="exps")
                for jt in range(bl0, bl0 + bln):
                    j0, kw = ktiles[jt]
                    b0 = (jt - bl0) * qw
                    nc.tensor.matmul(pst[:kw, b0:b0 + qw], bds[:qw, j0:j0 + kw],
                                     idbf[:qw, :qw], start=True, stop=False)
                    nc.tensor.matmul(pst[:kw, b0:b0 + qw],
                                     kb[mi][po:po + 64, j0:j0 + kw],
                                     qac[mi][po:po + 64, i0:i0 + qw],
                                     start=False, stop=True)
                nfull = sum(1 for jt in range(bl0, bl0 + bln)
                            if ktiles[jt][1] == 128)
                if nfull:
                    nc.scalar.activation(
                        exT[:, bl0 * qw:(bl0 + nfull) * qw],
                        pst[:, 0:nfull * qw], AF.Exp, scale=SCALE)
                for jt in range(bl0 + nfull, bl0 + bln):
                    j0, kw = ktiles[jt]
                    b0 = (jt - bl0) * qw
                    nc.scalar.activation(exT[:kw, jt * qw:jt * qw + qw],
                                         pst[:kw, b0:b0 + qw], AF.Exp, scale=SCALE)
            for jt in range(qi + 1):
                j0, kw = ktiles[jt]
                nc.tensor.matmul(pvh, vb[jt][:kw, hh * 65:hh * 65 + 65],
                                 exT[:kw, jt * qw:jt * qw + qw],
                                 start=(jt == 0), stop=(jt == qi))
            with nc.allow_low_precision(reason="softmax 1/den rounds to bf16"):
                nc.vector.reciprocal(deninv[mi][po:po + 1, i0:i0 + qw],
                                     pvh[64:65, :])
            vtmp = p.tile([64, 128], BF16, tag="dbb")[:, :qw]
            nc.vector.tensor_copy(vtmp, pvh[0:64, :])
            bps = psum.tile([128, 512], F32, tag="bd")[:64, :qw]
            nc.tensor.matmul(bps, ones_row_b[po:po + 1, :64],
                             deninv[mi][po:po + 1, i0:i0 + qw],
                             start=True, stop=True)
            nc.vector.tensor_tensor(voT[mi][po:po + 64, i0:i0 + qw],
                                    vtmp, bps, ALU.mult)

    # --- o-proj partial + AllReduce + residual + LN1 ---
    ob = p.tile([128, KD, NT], BF16, tag="arin", name="ob")
    for m in range(KD):
        for c0, cw in chunks:
            ps = psum.tile([128, 512], F32, tag="sc")[:, :cw]
            for kt in range(2):
                nc.tensor.matmul(ps, lw['woT'][kt][:, ts(m)], voT[kt][:, c0:c0 + cw],
                                 start=(kt == 0), stop=(kt == 1))
            nc.vector.tensor_copy(ob[:, m, c0:c0 + cw], ps)
    oar = _allreduce(nc, p, dram, ob, KD, NT, tag=f"o{lw['li']}_{lw['rep']}")
    xpre = []
    for m in range(KD):
        xp = p.tile([128, NT], F32, tag="big", name=f"xp{m}")
        nc.vector.tensor_tensor(xp[:], oar[:, m], XF[m][:], ALU.add)
        xpre.append(xp)
    XF, XB = _ln(nc, p, psum, xpre, NT, lw['g1'], lw['bb1'])

    # --- FFN ---
    hb = []
    for m in range(NKI):
        hbt = p.tile([128, NT], BF16, tag="hb", name=f"hb{m}")
        for c0, cw in chunks:
            ps = psum.tile([128, 512], F32, tag="sc")[:, :cw]
            for kd in range(KD):
                nc.tensor.matmul(ps, lw['w1T'][kd][:, ts(m)], XB[kd][:, c0:c0 + cw],
                                 start=(kd == 0), stop=(kd == KD - 1))
            nc.scalar.activation(hbt[:, c0:c0 + cw], ps, AF.Relu,
                                 bias=lw['fb1'][:, m:m + 1])
        hb.append(hbt)
    fb = p.tile([128, KD, NT], BF16, tag="arin", name="fb")
    for m in range(KD):
        for c0, cw in chunks:
            ps = psum.tile([128, 512], F32, tag="sc")[:, :cw]
            for kt in range(NKI):
                nc.tensor.matmul(ps, lw['w2T'][kt][:, ts(m)], hb[kt][:, c0:c0 + cw],
                                 start=(kt == 0), stop=(kt == NKI - 1))
            nc.vector.tensor_copy(fb[:, m, c0:c0 + cw], ps)
    far = _allreduce(nc, p, dram, fb, KD, NT, tag=f"f{lw['li']}_{lw['rep']}")
    xpre2 = []
    for m in range(KD):
        xp = p.tile([128, NT], F32, tag="big", name=f"xq{m}")
        for c0, cw in chunks:
            t1 = p.tile([128, 512], F32, tag="lnt")[:, :cw]
            nc.scalar.activation(t1, far[:, m, c0:c0 + cw], AF.Identity,
                                 bias=lw['fb2'][:, m:m + 1])
            nc.vector.tensor_tensor(xp[:, c0:c0 + cw], t1,
                                    XF[m][:, c0:c0 + cw], ALU.add)
        xpre2.append(xp)
    return _ln(nc, p, psum, xpre2, NT, lw['g2'], lw['bb2'])


def build_program():
    nc = bacc.Bacc("TRN2", target_bir_lowering=False, debug=False, num_devices=N_CORES)
    d = {}

    def di(name, shape, dt):
        d[name] = nc.dram_tensor(name, shape, dt, kind="ExternalInput")

    di("wqkvT", [L, D, 3 * QC], BF16)
    di("wrkT", [L, D, QC], BF16)
    di("woT", [L, QC, D], BF16)
    di("w1T", [L, D, DIO], BF16)
    di("w2T", [L, DIO, D], BF16)
    di("biases", [L, 128, 28], F32)
    di("rwb", [QC], F32)
    di("rrb", [QC], F32)
    di("wemb", [V, D], BF16)
    di("onehotT", [V, T], BF16)
    di("sinTd", [D, T], BF16)
    di("idbf", [128, 128], BF16)
    di("idf", [128, 128], F32)
    di("wpool", [T, SP], BF16)
    di("nullv", [D], F32)
    di("gd", [D], F32)
    di("bdn", [D], F32)
    di("uup", [SP, T], BF16)
    di("finT", [D, V // 2], BF16)
    di("fbn", [V // 2], F32)
    logits = nc.dram_tensor("logits", [T, V // 2], F32, kind="ExternalOutput")

    with tile.TileContext(nc) as tc:
        import itertools
        _ctr = itertools.count()

        class NP:
            def __init__(self, pool):
                self.pool = pool

            def tile(self, shape, dt, tag=None, name=None):
                if name is None:
                    name = f"{tag}_{next(_ctr)}"
                return self.pool.tile(shape, dt, tag=tag, name=name)

        with tc.tile_pool(name="p", bufs=2) as p_r, \
             tc.tile_pool(name="pbig", bufs=4) as pbig_r, \
             tc.tile_pool(name="px", bufs=4) as px_r, \
             tc.tile_pool(name="pr", bufs=4) as pr_r, \
             tc.tile_pool(name="ph", bufs=8) as ph_r, \
             tc.tile_pool(name="pw", bufs=2) as pw_r, \
             tc.tile_pool(name="pw1", bufs=2) as pw1_r, \
             tc.tile_pool(name="pw2", bufs=1) as pw2_r, \
             tc.tile_pool(name="pc", bufs=1) as pc_r, \
             tc.tile_pool(name="psum", bufs=2, space="PSUM") as psum_r, \
             tc.tile_pool(name="psum1", bufs=1, space="PSUM") as psum1_r, \
             tc.tile_pool(name="dram", bufs=1, space="DRAM") as dram_r:
            p, pbig, px, pr, ph, pw, pw1, pw2, pc = (NP(x) for x in
                                            (p_r, pbig_r, px_r, pr_r, ph_r,
                                             pw_r, pw1_r, pw2_r, pc_r))
            psum_, psum1_, dram = NP(psum_r), NP(psum1_r), NP(dram_r)

            # pool router: route tags to pools with the right bufs counts
            class P:
                def tile(self, shape, dt, tag=None, name=None):
                    if tag in ("XF", "XB"):
                        return px.tile(shape, dt, tag=tag, name=name)
                    if tag in ("big", "residF"):
                        return pbig.tile(shape, dt, tag=tag, name=name)
                    if tag in ("hb", "vb"):
                        return ph.tile(shape, dt, tag=tag, name=name)
                    if tag == "arin":
                        return pw2.tile(shape, dt, tag=tag, name=name)
                    if tag in ("arow", "brow", "mean", "var", "rv", "msq", "dro"):
                        return pc.tile(shape, dt, tag=tag, name=name)
                    if tag is not None and tag.startswith("c_"):
                        return pc.tile(shape, dt, tag=tag, name=name)
                    return p.tile(shape, dt, tag=tag, name=name)
            pp = P()

            class PS:
                def tile(self, shape, dt, tag=None, name=None):
                    if tag in ("st1", "exps"):
                        return psum1_.tile(shape, dt, tag=tag, name=name)
                    return psum_.tile(shape, dt, tag=tag, name=name)
            pps = PS()
            pools = (pp, pps, dram)

            consts = {}
            idbf = pc.tile([128, 128], BF16, tag="c_idbf")
            nc.sync.dma_start(idbf[:], d["idbf"].ap())
            idf = None  # unused
            ones_b = pc.tile([128, 1], BF16, tag="c_ones")
            nc.gpsimd.memset(ones_b[:], 1.0)
            ones_row = pc.tile([1, 128], F32, tag="c_onesr")
            nc.gpsimd.memset(ones_row[:], 1.0)
            ones_row_b = pc.tile([65, 128], BF16, tag="c_onesrb")
            nc.gpsimd.memset(ones_row_b[:], 1.0)
            _ln.ones_b = ones_b
            _ln.ones_row = ones_row
            _ln.ones_row_b = ones_row_b
            sinT_t = pc.tile([128, KD, T], BF16, tag="c_sin")
            nc.sync.dma_start(sinT_t[:],
                              d["sinTd"].ap().rearrange("(a q) t -> q a t", q=128))
            sinT = [sinT_t[:, k] for k in range(KD)]
            rwb = pc.tile([128, 2], F32, tag="c_rwb")
            nc.sync.dma_start(rwb[:], d["rwb"].ap().rearrange("(a q) -> q a", q=128))
            rrb = pc.tile([128, 2], F32, tag="c_rrb")
            nc.sync.dma_start(rrb[:], d["rrb"].ap().rearrange("(a q) -> q a", q=128))
            consts.update(idbf=idbf, idf=idf, sinT=sinT, rwb=rwb, rrb=rrb)

            def load_layer(li, rep):
                lw = {'li': li, 'rep': rep}
                wqkv = pw.tile([128, KD, 3 * QC], BF16, tag="w_qkv")
                nc.sync.dma_start(
                    wqkv[:], d["wqkvT"].ap()[li].rearrange("(a q) o -> q a o", q=128))
                lw['wqkvT'] = [wqkv[:, k] for k in range(KD)]
                wrk = pw.tile([128, KD, QC], BF16, tag="w_rk")
                nc.sync.dma_start(
                    wrk[:], d["wrkT"].ap()[li].rearrange("(a q) o -> q a o", q=128))
                lw['wrkT'] = [wrk[:, k] for k in range(KD)]
                wo = pw.tile([128, 2, D], BF16, tag="w_o")
                nc.sync.dma_start(
                    wo[:], d["woT"].ap()[li].rearrange("(a q) o -> q a o", q=128))
                lw['woT'] = [wo[:, k] for k in range(2)]
                w1 = pw1.tile([128, KD, DIO], BF16, tag="w_1")
                nc.sync.dma_start(
                    w1[:], d["w1T"].ap()[li].rearrange("(a q) o -> q a o", q=128))
                lw['w1T'] = [w1[:, k] for k in range(KD)]
                w2 = pw2.tile([128, NKI, D], BF16, tag="w_2")
                nc.sync.dma_start(
                    w2[:], d["w2T"].ap()[li].rearrange("(a q) o -> q a o", q=128))
                lw['w2T'] = [w2[:, k] for k in range(NKI)]
                bt = pw.tile([128, 28], F32, tag="w_b")
                nc.sync.dma_start(bt[:], d["biases"].ap()[li])
                lw['fb1'] = bt[:, 0:8]
                lw['fb2'] = bt[:, 8:12]
                lw['g1'] = bt[:, 12:16]
                lw['bb1'] = bt[:, 16:20]
                lw['g2'] = bt[:, 20:24]
                lw['bb2'] = bt[:, 24:28]
                return lw

            for rep in range(REPS):
                # --- embedding (one-hot matmul) ---
                wembt = pp.tile([128, 2, D], BF16, tag="c_wembt", name="wembt")
                nc.sync.dma_start(
                    wembt[:], d["wemb"].ap().rearrange("(a q) e -> q a e", q=128))
                ohs = []
                for c in range(2):
                    oh = pp.tile([128, 2, 512], BF16, tag=f"c_oh{c}", name=f"oh{c}")
                    nc.sync.dma_start(
                        oh[:],
                        d["onehotT"].ap().rearrange("(a q) t -> q a t", q=128)
                        [:, :, c * 512:(c + 1) * 512])
                    ohs.append(oh)
                XF, XB = [], []
                for m in range(KD):
                    xf = pp.tile([128, T], F32, tag="XF", name=f"XF{m}")
                    xb = pp.tile([128, T], BF16, tag="XB", name=f"XB{m}")
                    for c in range(2):
                        ps = pps.tile([128, 512], F32, tag="sc")
                        for vk in range(2):
                            nc.tensor.matmul(ps, wembt[:, vk, ts(m)], ohs[c][:, vk],
                                             start=(vk == 0), stop=(vk == 1))
                        nc.vector.tensor_copy(xf[:, c * 512:(c + 1) * 512], ps)
                        nc.scalar.copy(xb[:, c * 512:(c + 1) * 512], ps)
                    XF.append(xf)
                    XB.append(xb)

                for li in range(STAGES[0]):
                    XF, XB = _layer(nc, pools, load_layer(li, rep), XF, XB, T, consts)

                residF = []
                for m in range(KD):
                    r = pbig.tile([128, T], BF16, tag="residF", name=f"res{m}")
                    nc.vector.tensor_copy(r[:], XF[m][:])
                    residF.append(r)

                # --- downsample ---
                XN = []
                for tt in range(T // 128):
                    xn = pp.tile([128, D], BF16, tag="hb", name=f"xn{tt}")
                    for m in range(KD):
                        pt = pps.tile([128, 128], BF16, tag="sc")
                        nc.tensor.transpose(pt[:], XB[m][:, ts(tt)], idbf)
                        nc.vector.tensor_copy(xn[:, ts(m)], pt[:])
                    XN.append(xn)
                wpool = [pp.tile([128, SP], BF16, tag="vb", name=f"pl{tt}")
                         for tt in range(T // 128)]
                for tt in range(T // 128):
                    nc.sync.dma_start(wpool[tt][:], d["wpool"].ap()[ts(tt), :])
                nullv = pc.tile([128, KD], F32, tag="c_null")
                nc.sync.dma_start(nullv[:], d["nullv"].ap().rearrange("(a q) -> q a", q=128))
                spre = []
                for m in range(KD):
                    sp_ = pbig.tile([128, SP], F32, tag="big", name=f"sp{m}")
                    ps = pps.tile([128, 512], F32, tag="sc")[:, :SP]
                    for tt in range(T // 128):
                        nc.tensor.matmul(ps, XN[tt][:, ts(m)], wpool[tt][:],
                                         start=(tt == 0), stop=(tt == T // 128 - 1))
                    nc.vector.tensor_copy(sp_[:], ps)
                    nc.vector.tensor_copy(sp_[:, 0:1], nullv[:, m:m + 1])
                    spre.append(sp_)
                gdt = pc.tile([128, KD], F32, tag="c_gd")
                nc.sync.dma_start(gdt[:], d["gd"].ap().rearrange("(a q) -> q a", q=128))
                bdt = pc.tile([128, KD], F32, tag="c_bd")
                nc.sync.dma_start(bdt[:], d["bdn"].ap().rearrange("(a q) -> q a", q=128))
                SXF, SXB = _ln(nc, pp, pps, spre, SP, gdt, bdt)

                for li in range(STAGES[0], STAGES[0] + STAGES[1]):
                    SXF, SXB = _layer(nc, pools, load_layer(li, rep), SXF, SXB, SP, consts)

                # --- upsample + residual ---
                stiles = ttl(SP)
                SN = []
                for st, (s0, sw) in enumerate(stiles):
                    sn = pp.tile([128, D], BF16, tag="hb", name=f"sn{st}")
                    for m in range(KD):
                        pt = pps.tile([128, 128], BF16, tag="sc")[:sw, :]
                        nc.tensor.transpose(pt, SXB[m][:, s0:s0 + sw], idbf)
                        nc.vector.tensor_copy(sn[:sw, ts(m)], pt)
                    SN.append(sn)
                uup = [pp.tile([128, T], BF16, tag="hb", name=f"uu{st}")
                       for st in range(len(stiles))]
                for st, (s0, sw) in enumerate(stiles):
                    nc.sync.dma_start(uup[st][:sw, :], d["uup"].ap()[s0:s0 + sw, :])
                XF2, XB2 = [], []
                for m in range(KD):
                    xf = pp.tile([128, T], F32, tag="XF", name=f"XF{m}")
                    xb = pp.tile([128, T], BF16, tag="XB", name=f"XB{m}")
                    for c in range(2):
                        ps = pps.tile([128, 512], F32, tag="sc")
                        for st, (s0, sw) in enumerate(stiles):
                            nc.tensor.matmul(ps, SN[st][:sw, ts(m)],
                                             uup[st][:sw, c * 512:(c + 1) * 512],
                                             start=(st == 0),
                                             stop=(st == len(stiles) - 1))
                        nc.vector.tensor_tensor(xf[:, c * 512:(c + 1) * 512], ps,
                                                residF[m][:, c * 512:(c + 1) * 512], ALU.add)
                        nc.scalar.copy(xb[:, c * 512:(c + 1) * 512],
                                       xf[:, c * 512:(c + 1) * 512])
                    XF2.append(xf)
                    XB2.append(xb)
                XF, XB = XF2, XB2

                for li in range(STAGES[0] + STAGES[1], L):
                    XF, XB = _layer(nc, pools, load_layer(li, rep), XF, XB, T, consts)

                # --- final vocab projection (own half) ---
                finTt = pp.tile([128, KD, V // 2], BF16, tag="c_finT", name="finTt")
                nc.sync.dma_start(
                    finTt[:], d["finT"].ap().rearrange("(a q) v -> q a v", q=128))
                finT = [finTt[:, k] for k in range(KD)]
                fbn = pc.tile([1, V // 2], F32, tag="c_fbn")
                nc.sync.dma_start(fbn[:], d["fbn"].ap()[None, :])
                for tt in range(T // 128):
                    ps = pps.tile([128, 512], F32, tag="sc")[:, :V // 2]
                    for kd in range(KD):
                        nc.tensor.matmul(ps, XB[kd][:, ts(tt)], finT[kd][:],
                                         start=(kd == 0), stop=(kd == KD - 1))
                    nc.tensor.matmul(ps, ones_row, fbn[:], start=False, stop=True)
                    lo = p.tile([128, V // 2], F32, tag="louts")
                    nc.vector.tensor_copy(lo[:], ps)
                    nc.sync.dma_start(logits.ap()[ts(tt), :], lo[:])

    nc.compile()
    return nc


def host_inputs(inputs):
    bf = lambda x: np.ascontiguousarray(np.asarray(x, dtype=np.float32)).astype(ml_dtypes.bfloat16)
    f32 = lambda x: np.ascontiguousarray(np.asarray(x), dtype=np.float32)
    qkv_w = f32(inputs['qkv_w'])
    rk_w = f32(inputs['rk_w'])
    o_w = f32(inputs['o_w'])
    ff_w1 = f32(inputs['ff_w1'])
    ff_w2 = f32(inputs['ff_w2'])
    data = np.asarray(inputs['data'])
    bnd = np.asarray(inputs['boundaries_gt'])

    inv = 1.0 / (10000.0 ** (np.arange(0, D, 2, dtype=np.float32) / D))
    ang = np.arange(T, dtype=np.float32)[:, None] * inv[None, :]
    sin_tab = np.concatenate([np.sin(ang), np.cos(ang)], -1).astype(np.float32)
    eye = np.eye(128, dtype=np.float32)

    in_maps = []
    for c in range(N_CORES):
        b, h = c // 2, c % 2
        heads = list(range(h * HO, h * HO + HO))
        qr = np.concatenate([np.arange(g * DH, (g + 1) * DH) for g in heads])
        di_own = np.arange(h * DIO, (h + 1) * DIO)
        v_own = np.arange(h * (V // 2), (h + 1) * (V // 2))

        im = {}
        im['wqkvT'] = bf(np.stack([qkv_w[l][np.concatenate([qr, 512 + qr, 1024 + qr])].T
                                   for l in range(L)]))
        im['wrkT'] = bf(np.stack([rk_w[l][qr].T for l in range(L)]))
        im['woT'] = bf(np.stack([o_w[l][:, qr].T for l in range(L)]))
        im['w1T'] = bf(np.stack([ff_w1[l][di_own].T for l in range(L)]))
        im['w2T'] = bf(np.stack([ff_w2[l][:, di_own].T for l in range(L)]))
        fb1 = f32(np.asarray(inputs['ff_b1'])[:, di_own])        # [L, 1024]
        vecs = [f32(inputs['ff_b2']), f32(inputs['ln1_g']), f32(inputs['ln1_b']),
                f32(inputs['ln2_g']), f32(inputs['ln2_b'])]      # [L, 512] each
        # column q, slot a holds v[a*128+q] (matches "(a q) -> q a")
        bias_pack = np.zeros((L, 128, 28), np.float32)
        bias_pack[:, :, 0:8] = fb1.reshape(L, 8, 128).transpose(0, 2, 1)
        for i, v in enumerate(vecs):
            bias_pack[:, :, 8 + 4 * i:12 + 4 * i] = \
                v.reshape(L, 4, 128).transpose(0, 2, 1)
        im['biases'] = bias_pack
        im['rwb'] = f32(np.asarray(inputs['r_w_bias'])[heads].reshape(-1))
        im['rrb'] = f32(np.asarray(inputs['r_r_bias'])[heads].reshape(-1))
        im['wemb'] = bf(inputs['word_emb'])
        oh = np.zeros((V, T), np.float32)
        oh[data[:, b], np.arange(T)] = 1.0
        im['onehotT'] = bf(oh)
        im['sinTd'] = bf(sin_tab.T)
        im['idbf'] = bf(eye)
        im['idf'] = f32(eye)
        hard = bnd[:, b].astype(np.float32)
        seg = np.cumsum(hard) - hard
        ind = (seg[:, None] == np.arange(S)).astype(np.float32)
        wmat = ind / (ind.sum(0, keepdims=True) + 1e-9)
        wp_ = np.zeros((T, SP), np.float32)     # cols S+1..SP stay zero pad
        wp_[:, 1:1 + S] = wmat
        im['wpool'] = bf(wp_)
        im['nullv'] = f32(np.asarray(inputs['null_group']).reshape(-1))
        im['gd'] = f32(inputs['down_ln_g'])
        im['bdn'] = f32(inputs['down_ln_b'])
        segU = np.clip(np.cumsum(hard).astype(np.int64), 0, S)
        uu = np.zeros((SP, T), np.float32)
        uu[segU, np.arange(T)] = 1.0
        im['uup'] = bf(uu)
        im['finT'] = bf(f32(inputs['final_w'])[v_own].T)
        im['fbn'] = f32(np.asarray(inputs['final_b'])[v_own])
        in_maps.append(im)
    return in_maps


_NC_CACHE = {}


def get_program():
    if 'nc' not in _NC_CACHE:
        _NC_CACHE['nc'] = build_program()
    return _NC_CACHE['nc']


def assemble_output(results) -> np.ndarray:
    out = np.zeros((T, B, V), np.float32)
    for c in range(N_CORES):
        b, h = c // 2, c % 2
        out[:, b, h * (V // 2):(h + 1) * (V // 2)] = results[c]['logits']
    return out


def kernel(**inputs) -> np.ndarray:
    nc = get_program()
    in_maps = host_inputs(inputs)
    res = run_bass_kernel_spmd(nc, in_maps, core_ids=list(range(N_CORES)), trace=False)
    return assemble_output(res.results)

